# revision 1
# baseline (speedup 1.0000x reference)
"""Trainium2 Bass kernel for DeLanNet inverse dynamics.

out = tau_m + c1 + c2 + g   where per batch element (q, v=qDot, a2=qDDot):
  L = lower-tri from two MLPs on q, H = L L^T
  tau = L (L^T a2)
  c1  = 2 * (Dd p + Do u),  p = v*w, w = L^T v, u = outer-gathered v_i*w_j
  c2  = L alpha + A w,      A = dL/dq . v  (directional derivative)
  g   = MLP_g(q)

Key trick: the per-element Jacobians Dd[k,m] = d h_ld[m]/dq_k and
Do[k,n] = d h_lo[n]/dq_k are computed as a single matmul against
host-precomputed constant matrices:
  Dd = (1 - a_d^2) @ Gd  with Gd[h, m*7+k] = Wd1[k,h]*Wd2[h,m]
(1-sq)@Gd is folded as colsum(Gd) - sq@Gd, the colsum going into a
constant bias row added when copying PSUM->SBUF.

Sharding: pure data parallel over 8 cores (4096 batch elements each),
MLP weights/constants replicated.

Layout: "F-hidden": hidden activations live as [128 hid-chunk, Nb batch]
tiles so they are directly usable as matmul lhsT (stationary operand)
for the K=512 contractions, with no on-chip transposes anywhere.
"""

import numpy as np

import concourse.bass as bass
import concourse.bacc as bacc
import concourse.mybir as mybir
import concourse.tile as tile
from concourse.bass_utils import run_bass_kernel_spmd

DOF = 7
HID = 512
B_FULL = 32768
N_CORES = 8
B_CORE = B_FULL // N_CORES  # 4096

F32 = mybir.dt.float32
BF16 = mybir.dt.bfloat16

# ---- tunables ----
import os
NB = int(os.environ.get("K_NB", "1024"))   # batch group size (multiple of 128)
DT_Z = os.environ.get("K_DT_Z", "f32r")    # first-layer matmul: bf16 | f32r | f32
DT_C = os.environ.get("K_DT_C", "bf16")    # contraction matmuls: bf16 | f32
A_BUFS = int(os.environ.get("K_A_BUFS", "26"))
SQ_BUFS = int(os.environ.get("K_SQ_BUFS", "18"))
Z_BUFS = int(os.environ.get("K_Z_BUFS", "2"))
S_BUFS = int(os.environ.get("K_S_BUFS", "4"))
N_SQ_ACT = int(os.environ.get("K_N_SQ_ACT", "0"))   # sq chunks (of 8) done on ScalarE
COPY_ACT = int(os.environ.get("K_COPY_ACT", "1"))   # every COPY_ACT+1'th S-copy on DVE, rest ScalarE; 0=all DVE
SS_BUFS = int(os.environ.get("K_SS_BUFS", "2"))     # Ssb (group smalls) bufs
SQ_POW = int(os.environ.get("K_SQ_POW", "0"))       # sq via tensor_scalar pow-2 (single-src)
DT_T = os.environ.get("K_DT_T", "f32")              # mul-temp dtype: f32 | bf16
DT_MM = DT_C  # host-side dtype for contraction consts

_pairs_cm = [(i, j) for j in range(DOF - 1) for i in range(j + 1, DOF)]
_grp_base = [0]
for _j in range(6):
    _grp_base.append(_grp_base[-1] + (6 - _j))


def _host_constants(Wd1, bd1, Wd2, bd2, Wo1, bo1, Wo2, bo2, Wg1, bg1, Wg2, bg2):
    TI, TJ = np.tril_indices(DOF, -1)
    orig_idx = np.array(
        [int(np.where((TI == i) & (TJ == j))[0][0]) for (i, j) in _pairs_cm]
    )
    Wo2_cm = Wo2[:, orig_idx]
    bo2_cm = bo2[orig_idx]

    W1cat = np.concatenate([Wd1, Wo1, Wg1], axis=1).astype(np.float32)  # [7,1536]
    b1cat = np.concatenate([bd1, bo1, bg1]).astype(np.float32)          # [1536]

    WL_d = np.zeros((HID, 49), np.float32)
    for m in range(DOF):
        WL_d[:, m * 7 + m] = Wd2[:, m]
    WL_o = np.zeros((HID, 49), np.float32)
    for n, (i, j) in enumerate(_pairs_cm):
        WL_o[:, i * 7 + j] = Wo2_cm[:, n]

    Gd_n = np.zeros((HID, 49), np.float32)   # negated Gd
    for m in range(DOF):
        for k in range(DOF):
            Gd_n[:, m * 7 + k] = -Wd1[k, :] * Wd2[:, m]
    Go_n = np.zeros((HID, 147), np.float32)  # negated Go
    for n in range(21):
        for k in range(DOF):
            Go_n[:, n * 7 + k] = -Wo1[k, :] * Wo2_cm[:, n]

    bias_row = np.zeros(252, np.float32)
    for m in range(DOF):
        bias_row[m * 7 + m] += bd2[m]
    for n, (i, j) in enumerate(_pairs_cm):
        bias_row[i * 7 + j] += bo2_cm[n]
    bias_row[49:98] = -Gd_n.sum(axis=0)
    bias_row[98:245] = -Go_n.sum(axis=0)
    bias_row[245:252] = bg2

    def chunkmaj(M):  # [512, N] -> [128, 4, N] with [p, c, n] = M[c*128+p, n]
        N = M.shape[1]
        return M.reshape(4, 128, N).transpose(1, 0, 2).copy()

    import ml_dtypes
    np_c = np.float32 if DT_C == "f32" else ml_dtypes.bfloat16
    np_z = np.float32 if DT_Z != "bf16" else ml_dtypes.bfloat16

    return {
        "W1cat": W1cat.astype(np_z),                       # [7, 1536]
        "b1sb": b1cat.reshape(12, 128).T.copy(),           # [128, 12] f32
        "WLd": chunkmaj(WL_d).astype(np_c),                # [128, 4, 49]
        "WLo": chunkmaj(WL_o).astype(np_c),
        "Gdn": chunkmaj(Gd_n).astype(np_c),
        "Gon": chunkmaj(Go_n).astype(np_c),                # [128, 4, 147]
        "Wg2c": chunkmaj(Wg2.astype(np.float32)).astype(np_c),  # [128, 4, 7]
        "bias_bc": np.broadcast_to(bias_row, (128, 252)).copy(),  # [128,252] f32
    }, np_z


def build_bass():
    dt_c = F32 if DT_C == "f32" else BF16
    F32R = mybir.dt.float32r
    dt_z = {"f32": F32, "f32r": F32R, "bf16": BF16}[DT_Z]

    def zview(ap):  # matmul-operand view for the Z path
        return ap

    nc = bacc.Bacc("TRN2", target_bir_lowering=False, debug=False)

    x_s = nc.dram_tensor("x_s", [B_CORE, 21], F32, kind="ExternalInput").ap()
    xqT = nc.dram_tensor("xqT", [DOF, B_CORE], dt_z, kind="ExternalInput").ap()
    W1cat_d = nc.dram_tensor("W1cat", [DOF, 1536], dt_z, kind="ExternalInput").ap()
    b1sb_d = nc.dram_tensor("b1sb", [128, 12], F32, kind="ExternalInput").ap()
    WLd_d = nc.dram_tensor("WLd", [128, 4, 49], dt_c, kind="ExternalInput").ap()
    WLo_d = nc.dram_tensor("WLo", [128, 4, 49], dt_c, kind="ExternalInput").ap()
    Gdn_d = nc.dram_tensor("Gdn", [128, 4, 49], dt_c, kind="ExternalInput").ap()
    Gon_d = nc.dram_tensor("Gon", [128, 4, 147], dt_c, kind="ExternalInput").ap()
    Wg2_d = nc.dram_tensor("Wg2c", [128, 4, 7], dt_c, kind="ExternalInput").ap()
    bias_d = nc.dram_tensor("bias_bc", [128, 252], F32, kind="ExternalInput").ap()
    out_s = nc.dram_tensor("out_s", [B_CORE, DOF], F32, kind="ExternalOutput").ap()

    dt_t = F32 if DT_T == "f32" else BF16
    NG = B_CORE // NB          # groups
    NS = NB // 128             # subtiles per group
    MUL = mybir.AluOpType.mult
    ADD = mybir.AluOpType.add

    with tile.TileContext(nc) as tc:
        import contextlib
        ctx = contextlib.ExitStack()
        with ctx:
            consts = ctx.enter_context(tc.tile_pool(name="consts", bufs=1))
            apool = ctx.enter_context(tc.tile_pool(name="apool", bufs=A_BUFS))
            sqpool = ctx.enter_context(tc.tile_pool(name="sqpool", bufs=SQ_BUFS))
            xq_pool = ctx.enter_context(tc.tile_pool(name="xqp", bufs=2))
            zpool = ctx.enter_context(tc.tile_pool(name="zp", bufs=Z_BUFS, space="PSUM"))
            spool = ctx.enter_context(tc.tile_pool(name="sp", bufs=S_BUFS, space="PSUM"))
            smalls = ctx.enter_context(tc.tile_pool(name="smalls", bufs=SS_BUFS))
            stmp = ctx.enter_context(tc.tile_pool(name="stmp", bufs=2))
            souts = ctx.enter_context(tc.tile_pool(name="souts", bufs=2))

            # ---- constants into SBUF ----
            W1_sb = consts.tile([DOF, 1536], dt_z)
            nc.sync.dma_start(out=W1_sb, in_=W1cat_d)
            b1_sb = consts.tile([128, 12], F32)
            nc.sync.dma_start(out=b1_sb, in_=b1sb_d)
            WLd_sb = consts.tile([128, 4, 49], dt_c)
            nc.sync.dma_start(out=WLd_sb, in_=WLd_d)
            WLo_sb = consts.tile([128, 4, 49], dt_c)
            nc.sync.dma_start(out=WLo_sb, in_=WLo_d)
            Gdn_sb = consts.tile([128, 4, 49], dt_c)
            nc.sync.dma_start(out=Gdn_sb, in_=Gdn_d)
            Gon_sb = consts.tile([128, 4, 147], dt_c)
            nc.sync.dma_start(out=Gon_sb, in_=Gon_d)
            Wg2_sb = consts.tile([128, 4, 7], dt_c)
            nc.sync.dma_start(out=Wg2_sb, in_=Wg2_d)
            bias_sb = consts.tile([128, 252], F32)
            nc.sync.dma_start(out=bias_sb, in_=bias_d)

            Az = consts.tile([128, NS, 49], F32)  # dense A, zeros persist
            nc.vector.memset(Az, 0.0)

            def emit_z(gidx):
                b0 = gidx * NB

                # xqT slice for this group: [7, NB]
                xq_sb = xq_pool.tile([DOF, NB], dt_z)
                nc.sync.dma_start(out=xq_sb, in_=xqT[:, b0 : b0 + NB])

                # ---- phase 1: Z = W1^T.T @ xq -> tanh -> a (bf16), sq ----
                a_tiles = []
                sq_tiles = []
                for c in range(12):
                    zt = zpool.tile([128, NB], F32)  # PSUM
                    n512 = NB // 512
                    for jj in range(n512):
                        nc.tensor.matmul(
                            zt[:, jj * 512 : (jj + 1) * 512],
                            lhsT=zview(W1_sb[:, c * 128 : (c + 1) * 128]),
                            rhs=zview(xq_sb[:, jj * 512 : (jj + 1) * 512]),
                            start=True,
                            stop=True,
                        )
                    at = apool.tile([128, NB], dt_c, tag="a")
                    nc.scalar.activation(
                        at, zt, mybir.ActivationFunctionType.Tanh,
                        bias=b1_sb[:, c : c + 1], scale=1.0,
                    )
                    a_tiles.append(at)
                    if c < 8:
                        st = sqpool.tile([128, NB], dt_c, tag="sq")
                        if c < N_SQ_ACT:
                            nc.scalar.activation(
                                st, at, mybir.ActivationFunctionType.Square)
                        elif SQ_POW:
                            nc.vector.tensor_scalar(
                                st, at, 2.0, None, mybir.AluOpType.pow)
                        else:
                            nc.vector.tensor_mul(st, at, at)
                        sq_tiles.append(st)

                # prefetch v / qDDot slices for the smalls phase
                vt = souts.tile([128, NS, 7], F32, tag="vt")
                a2t = souts.tile([128, NS, 7], F32, tag="a2t")
                xg = x_s[b0 : b0 + NB, :].rearrange("(t p) f -> p t f", p=128)
                nc.sync.dma_start(out=vt, in_=xg[:, :, 7:14])
                nc.sync.dma_start(out=a2t, in_=xg[:, :, 14:21])
                return a_tiles, sq_tiles, vt, a2t

            def emit_contraction(gidx, a_tiles, sq_tiles):
                # ---- phase 2: per-subtile contractions into PSUM [128, 252] ----
                Ssb = smalls.tile([128, NS, 252], F32, tag="S")
                for s in range(NS):
                    bs = slice(s * 128, (s + 1) * 128)
                    ps = spool.tile([128, 252], F32)
                    # L: cols 0:49  (a_d chunks then a_o chunks)
                    for c4 in range(4):
                        nc.tensor.matmul(
                            ps[:, 0:49], lhsT=a_tiles[c4][:, bs],
                            rhs=WLd_sb[:, c4, :], start=(c4 == 0), stop=False,
                        )
                    for c4 in range(4):
                        nc.tensor.matmul(
                            ps[:, 0:49], lhsT=a_tiles[4 + c4][:, bs],
                            rhs=WLo_sb[:, c4, :], start=False, stop=(c4 == 3),
                        )
                    # Dd: cols 49:98  (sq_d)
                    for c4 in range(4):
                        nc.tensor.matmul(
                            ps[:, 49:98], lhsT=sq_tiles[c4][:, bs],
                            rhs=Gdn_sb[:, c4, :], start=(c4 == 0), stop=(c4 == 3),
                        )
                    # Do: cols 98:245  (sq_o)
                    for c4 in range(4):
                        nc.tensor.matmul(
                            ps[:, 98:245], lhsT=sq_tiles[4 + c4][:, bs],
                            rhs=Gon_sb[:, c4, :], start=(c4 == 0), stop=(c4 == 3),
                        )
                    # g: cols 245:252  (a_g)
                    for c4 in range(4):
                        nc.tensor.matmul(
                            ps[:, 245:252], lhsT=a_tiles[8 + c4][:, bs],
                            rhs=Wg2_sb[:, c4, :], start=(c4 == 0), stop=(c4 == 3),
                        )
                    # PSUM -> SBUF with the constant bias row added
                    nc.vector.tensor_add(Ssb[:, s, :], ps, bias_sb)
                return Ssb

            def emit_smalls(gidx, Ssb, vt, a2t):
                # ---- phase 3: smalls over [128, NS, *] ----
                b0 = gidx * NB
                Lv = Ssb[:, :, 0:49].rearrange("p t (i j) -> p t i j", j=7)
                Dd = Ssb[:, :, 49:98].rearrange("p t (m k) -> p t m k", k=7)
                Do = Ssb[:, :, 98:245].rearrange("p t (n k) -> p t n k", k=7)
                g_t = Ssb[:, :, 245:252]

                def bcast_inner(ap7):  # [128,NS,7] -> [128,NS,7(idx),7(bcast)]
                    return ap7.unsqueeze(3).broadcast_to((128, NS, 7, 7))

                def bcast_outer(ap7):  # [128,NS,7] -> [128,NS,7(bcast),7(idx)]
                    return ap7.unsqueeze(2).broadcast_to((128, NS, 7, 7))

                t49 = stmp.tile([128, NS, 7, 7], dt_t, tag="t49")
                # w = L^T v
                w_t = souts.tile([128, NS, 7], F32, tag="w")
                nc.vector.tensor_mul(t49, Lv, bcast_inner(vt))
                nc.vector.reduce_sum(
                    w_t, t49.rearrange("p t i j -> p t j i"), axis=mybir.AxisListType.X
                )
                # t1 = L^T a2 ; tau = L t1
                t49b = stmp.tile([128, NS, 7, 7], dt_t, tag="t49")
                t1_t = souts.tile([128, NS, 7], F32, tag="t1")
                nc.vector.tensor_mul(t49b, Lv, bcast_inner(a2t))
                nc.vector.reduce_sum(
                    t1_t, t49b.rearrange("p t i j -> p t j i"), axis=mybir.AxisListType.X
                )
                t49c = stmp.tile([128, NS, 7, 7], dt_t, tag="t49")
                tau_t = souts.tile([128, NS, 7], F32, tag="tau")
                nc.vector.tensor_mul(t49c, Lv, bcast_outer(t1_t))
                nc.vector.reduce_sum(tau_t, t49c, axis=mybir.AxisListType.X)
                # p = v*w
                p_t = souts.tile([128, NS, 7], F32, tag="p")
                nc.vector.tensor_mul(p_t, vt, w_t)
                # u: col-major gathered products
                u_t = souts.tile([128, NS, 21], F32, tag="u")
                for j in range(6):
                    nb0 = _grp_base[j]
                    cnt = 6 - j
                    nc.vector.tensor_mul(
                        u_t[:, :, nb0 : nb0 + cnt],
                        vt[:, :, j + 1 : 7],
                        w_t[:, :, j : j + 1].broadcast_to((128, NS, cnt)),
                    )
                # c1d = Dd^T(p), c1o = Do^T(u)  (sum over m / n)
                t49d = stmp.tile([128, NS, 7, 7], dt_t, tag="t49")
                c1d_t = souts.tile([128, NS, 7], F32, tag="c1d")
                nc.vector.tensor_mul(t49d, Dd, bcast_inner(p_t))
                nc.vector.reduce_sum(
                    c1d_t, t49d.rearrange("p t m k -> p t k m"), axis=mybir.AxisListType.X
                )
                t147 = stmp.tile([128, NS, 21, 7], dt_t, tag="t147")
                c1o_t = souts.tile([128, NS, 7], F32, tag="c1o")
                nc.vector.tensor_mul(
                    t147, Do, u_t[:].unsqueeze(3).broadcast_to((128, NS, 21, 7))
                )
                nc.vector.reduce_sum(
                    c1o_t, t147.rearrange("p t n k -> p t k n"), axis=mybir.AxisListType.X
                )
                # dd = Dd v (sum over k), do = Do v
                t49e = stmp.tile([128, NS, 7, 7], dt_t, tag="t49")
                dd_t = souts.tile([128, NS, 7], F32, tag="dd")
                nc.vector.tensor_mul(t49e, Dd, bcast_outer(vt))
                nc.vector.reduce_sum(dd_t, t49e, axis=mybir.AxisListType.X)
                t147b = stmp.tile([128, NS, 21, 7], dt_t, tag="t147")
                do_t = souts.tile([128, NS, 21], F32, tag="do")
                nc.vector.tensor_mul(
                    t147b, Do, vt.unsqueeze(2).broadcast_to((128, NS, 21, 7))
                )
                nc.vector.reduce_sum(do_t, t147b, axis=mybir.AxisListType.X)
                # alpha
                ad_t = souts.tile([128, NS, 7], F32, tag="ad")
                nc.vector.tensor_mul(ad_t, dd_t, vt)
                t2_t = souts.tile([128, NS, 21], F32, tag="t2")
                al_t = souts.tile([128, NS, 6], F32, tag="al")
                for j in range(6):
                    nb0 = _grp_base[j]
                    cnt = 6 - j
                    nc.vector.tensor_mul(
                        t2_t[:, :, nb0 : nb0 + cnt],
                        do_t[:, :, nb0 : nb0 + cnt],
                        vt[:, :, j + 1 : 7],
                    )
                for j in range(6):
                    nb0 = _grp_base[j]
                    cnt = 6 - j
                    nc.vector.reduce_sum(
                        al_t[:, :, j : j + 1],
                        t2_t[:, :, nb0 : nb0 + cnt],
                        axis=mybir.AxisListType.X,
                    )
                alpha_t = souts.tile([128, NS, 7], F32, tag="alpha")
                nc.vector.tensor_add(
                    alpha_t[:, :, 0:6], ad_t[:, :, 0:6], al_t[:, :, 0:6]
                )
                nc.vector.tensor_copy(alpha_t[:, :, 6:7], ad_t[:, :, 6:7])
                # c2a = L alpha
                t49f = stmp.tile([128, NS, 7, 7], dt_t, tag="t49")
                c2a_t = souts.tile([128, NS, 7], F32, tag="c2a")
                nc.vector.tensor_mul(t49f, Lv, bcast_outer(alpha_t))
                nc.vector.reduce_sum(c2a_t, t49f, axis=mybir.AxisListType.X)
                # build dense A (diag dd, lower do) in persistent zeroed Az
                diag_ap = bass.AP(
                    tensor=Az.tensor,
                    offset=Az.offset,
                    ap=[Az[:].ap[0], [49, NS], [8, 7]],
                )
                nc.vector.tensor_copy(diag_ap, dd_t)
                for j in range(6):
                    nb0 = _grp_base[j]
                    cnt = 6 - j
                    low_ap = bass.AP(
                        tensor=Az.tensor,
                        offset=Az.offset + (8 * j + 7),
                        ap=[Az[:].ap[0], [49, NS], [7, cnt]],
                    )
                    nc.vector.tensor_copy(low_ap, do_t[:, :, nb0 : nb0 + cnt])
                # c2b = A w
                t49g = stmp.tile([128, NS, 7, 7], dt_t, tag="t49")
                c2b_t = souts.tile([128, NS, 7], F32, tag="c2b")
                nc.vector.tensor_mul(
                    t49g,
                    Az[:].rearrange("p t (i j) -> p t i j", j=7),
                    bcast_outer(w_t),
                )
                nc.vector.reduce_sum(c2b_t, t49g, axis=mybir.AxisListType.X)
                # assemble: out = tau + c2a + c2b + g + 2*(c1d+c1o)
                o1 = souts.tile([128, NS, 7], F32, tag="o1")
                nc.vector.tensor_add(o1, tau_t, c2a_t)
                o2 = souts.tile([128, NS, 7], F32, tag="o2")
                nc.vector.tensor_add(o2, o1, c2b_t)
                o3 = souts.tile([128, NS, 7], F32, tag="o3")
                nc.vector.tensor_add(o3, o2, g_t)
                c1s = souts.tile([128, NS, 7], F32, tag="c1s")
                nc.vector.tensor_add(c1s, c1d_t, c1o_t)
                of = souts.tile([128, NS, 7], F32, tag="of")
                nc.vector.scalar_tensor_tensor(
                    of, in0=c1s, scalar=2.0, in1=o3, op0=MUL, op1=ADD
                )
                # store
                og = out_s[b0 : b0 + NB, :].rearrange("(t p) f -> p t f", p=128)
                nc.sync.dma_start(out=og, in_=of)

            # software-pipelined emission, depth controlled by K_PIPE:
            #   0: fully sequential per group
            #   1: Z(g+1) emitted before smalls(g)
            #   2: Z(g+1) before contraction(g), smalls lag 2
            PIPE = int(os.environ.get("K_PIPE", "0"))
            zs = {}
            cs = {}
            if PIPE == 0:
                for g in range(NG):
                    a_t, sq_t, vt_, a2t_ = emit_z(g)
                    Ssb_ = emit_contraction(g, a_t, sq_t)
                    emit_smalls(g, Ssb_, vt_, a2t_)
            elif PIPE == 1:
                for step in range(NG + 1):
                    if step < NG:
                        zs[step] = emit_z(step)
                    if step >= 1:
                        g2 = step - 1
                        a_t, sq_t, vt_, a2t_ = zs.pop(g2)
                        Ssb_ = emit_contraction(g2, a_t, sq_t)
                        emit_smalls(g2, Ssb_, vt_, a2t_)
            else:
                for step in range(NG + 2):
                    if step < NG:
                        zs[step] = emit_z(step)
                    if 1 <= step <= NG:
                        g2 = step - 1
                        a_t, sq_t, vt_, a2t_ = zs[g2]
                        cs[g2] = (emit_contraction(g2, a_t, sq_t), vt_, a2t_)
                    if 2 <= step <= NG + 1:
                        g3 = step - 2
                        Ssb_, vt_, a2t_ = cs.pop(g3)
                        emit_smalls(g3, Ssb_, vt_, a2t_)
                        zs.pop(g3, None)

    nc.compile()
    return nc


_CACHED = {}


def kernel(**inputs):
    x = np.asarray(inputs["x"], np.float32)
    consts, np_mm = _host_constants(
        *[np.asarray(inputs[k], np.float32) for k in (
            "Wd1", "bd1", "Wd2", "bd2", "Wo1", "bo1", "Wo2", "bo2",
            "Wg1", "bg1", "Wg2", "bg2")]
    )

    if "nc" not in _CACHED:
        _CACHED["nc"] = build_bass()
    nc = _CACHED["nc"]

    xqT_full = np.ascontiguousarray(x[:, 0:DOF].T).astype(np_mm)  # [7, B]

    in_maps = []
    for c in range(N_CORES):
        sl = slice(c * B_CORE, (c + 1) * B_CORE)
        m = {"x_s": np.ascontiguousarray(x[sl]),
             "xqT": np.ascontiguousarray(xqT_full[:, sl])}
        m.update(consts)
        in_maps.append(m)

    res = run_bass_kernel_spmd(nc, in_maps, core_ids=list(range(N_CORES)))
    outs = [res.results[c]["out_s"] for c in range(N_CORES)]
    return np.concatenate(outs, axis=0).astype(np.float32)


def _make_in_maps(inputs):
    x = np.asarray(inputs["x"], np.float32)
    consts, np_mm = _host_constants(
        *[np.asarray(inputs[k], np.float32) for k in (
            "Wd1", "bd1", "Wd2", "bd2", "Wo1", "bo1", "Wo2", "bo2",
            "Wg1", "bg1", "Wg2", "bg2")]
    )
    xqT_full = np.ascontiguousarray(x[:, 0:DOF].T).astype(np_mm)
    in_maps = []
    for c in range(N_CORES):
        sl = slice(c * B_CORE, (c + 1) * B_CORE)
        m = {"x_s": np.ascontiguousarray(x[sl]),
             "xqT": np.ascontiguousarray(xqT_full[:, sl])}
        m.update(consts)
        in_maps.append(m)
    return in_maps


def profile_once(inputs, tmpdir=None):
    """Run once with NTFF tracing; return device exec time in ns (or None)."""
    if "nc" not in _CACHED:
        _CACHED["nc"] = build_bass()
    nc = _CACHED["nc"]
    res = run_bass_kernel_spmd(
        nc, _make_in_maps(inputs), core_ids=list(range(N_CORES)),
        trace=True, tmpdir=tmpdir,
    )
    return res.exec_time_ns


def time_device(inputs, iters=20):
    """Best-effort device-time estimate: build the sharded jit once (same
    construction as bass2jax.run_bass_via_pjrt), pre-stage inputs on device,
    and report the min wall time of repeated dispatches."""
    import time

    import jax
    import jax.numpy as jnp
    from jax.sharding import Mesh, PartitionSpec
    from jax.experimental.shard_map import shard_map
    from concourse import bass2jax
    from concourse import mybir as mb

    bass2jax.install_neuronx_cc_hook()
    if "nc" not in _CACHED:
        _CACHED["nc"] = build_bass()
    nc = _CACHED["nc"]
    in_maps = _make_in_maps(inputs)

    partition_name = (
        nc.partition_id_tensor.name if nc.partition_id_tensor else None
    )
    in_names, out_names, out_avals, zero_outs = [], [], [], []
    for alloc in nc.m.functions[0].allocations:
        if not isinstance(alloc, mb.MemoryLocationSet):
            continue
        name = alloc.memorylocations[0].name
        if alloc.kind == "ExternalInput":
            if name != partition_name:
                in_names.append(name)
        elif alloc.kind == "ExternalOutput":
            out_names.append(name)
            shape = tuple(alloc.tensor_shape)
            dtype = mb.dt.np(alloc.dtype)
            out_avals.append(jax.core.ShapedArray(shape, dtype))
            zero_outs.append(np.zeros(shape, dtype))
    n_params = len(in_names)
    all_in = list(in_names) + list(out_names)
    if partition_name is not None:
        all_in.append(partition_name)

    def _body(*args):
        operands = list(args)
        if partition_name is not None:
            operands.append(bass2jax.partition_id_tensor())
        outs = bass2jax._bass_exec_p.bind(
            *operands,
            out_avals=tuple(out_avals),
            in_names=tuple(all_in),
            out_names=tuple(out_names),
            lowering_input_output_aliases=(),
            sim_require_finite=True,
            sim_require_nnan=True,
            nc=nc,
        )
        return tuple(outs)

    devices = jax.devices()[:N_CORES]
    mesh = Mesh(np.asarray(devices), ("core",))
    nin = n_params + len(zero_outs)
    sharded = jax.jit(
        shard_map(
            _body, mesh=mesh,
            in_specs=(PartitionSpec("core"),) * nin,
            out_specs=(PartitionSpec("core"),) * len(out_names),
            check_rep=False,
        ),
    )
    concat_in = [
        np.concatenate([np.asarray(in_maps[c][nm]) for c in range(N_CORES)], axis=0)
        for nm in in_names
    ]
    concat_zeros = [
        np.zeros((N_CORES * z.shape[0], *z.shape[1:]), z.dtype) for z in zero_outs
    ]
    sharding = jax.sharding.NamedSharding(mesh, PartitionSpec("core"))
    dev_in = [jax.device_put(a, sharding) for a in concat_in + concat_zeros]
    out = sharded(*dev_in)
    jax.block_until_ready(out)
    best = float("inf")
    for _ in range(iters):
        t0 = time.perf_counter()
        out = sharded(*dev_in)
        jax.block_until_ready(out)
        best = min(best, time.perf_counter() - t0)
    return best * 1e9

